# revision 53
# baseline (speedup 1.0000x reference)
"""Trainium2 Bass kernel for nn_ExpertsChooseBlock (experts-choose MoE block).

Sharding: pure data-parallel over batch B=8 across 8 NeuronCores (one batch
element per core, no collectives).  Per core:
  P1  streaming pass over x: residual init (DRAM->DRAM), LayerNorm1 stats +
      rescale -> xn staged to DRAM in bf16, x^T via exact-fp32 PE transposes,
      router logits (fp32, moving operand = Wr so free dim is E), per-group
      softmax -> probs staged to DRAM.
  P2  exact top-512 threshold per expert via gpsimd kth_largest + one
      masked-max chase; thresholds broadcast on-chip via PE transposes and
      selection matmuls (no DRAM round trips); index/gate compaction via
      sparse_gather; index replication via PE matmul.
  P3  attention branch: transpose-mode dma_gather of LN'd bf16 tokens
      (feature-major, nested-mask prefix only), ln gamma/beta fold, bf16
      qkv/scores/AV/proj matmuls, gate-scaled dma_scatter_add into out.
  P4  MLP branch: fp32 gather from updated out, LayerNorm2, bf16 W1/W2
      (resident in SBUF) with gelu, gate-scaled dma_scatter_add into out.
"""

import numpy as np

import concourse.bass as bass
import concourse.mybir as mybir
import concourse.tile as tile
from concourse import bacc
from concourse.bass_utils import run_bass_kernel_spmd

F32 = mybir.dt.float32
F32R = mybir.dt.float32r
BF16 = mybir.dt.bfloat16
FP8 = mybir.dt.float8e4
DR = mybir.MatmulPerfMode.DoubleRow
W8SCALE = 32.0
I16 = mybir.dt.int16
U32 = mybir.dt.uint32
AF = mybir.ActivationFunctionType
ALU = mybir.AluOpType
AX = mybir.AxisListType

B, N, D, E, HEADS, HID = 8, 2048, 768, 4, 12, 3072
CAP = 512
DH = 64
EPS = 1e-5
NT = N // 128          # 16 token tiles
KD = D // 128          # 6 feature tiles
KH = HID // 128        # 24 hidden tiles

DE = [D >> e for e in range(E)]            # [768, 384, 192, 96]
KDE = [(d + 127) // 128 for d in DE]       # [6, 3, 2, 1]
HIDE = [HID >> e for e in range(E)]        # [3072, 1536, 768, 384]
KHE = [h // 128 for h in HIDE]             # [24, 12, 6, 3]
DPAD = [768, 384, 256, 128]                # gather/scatter elem sizes (256B)
HEADS_E = []
for _e in range(E):
    hs, d = [], 0
    while d < DE[_e]:
        hs.append((d // DH, min(DH, DE[_e] - d)))
        d += DH
    HEADS_E.append(hs)

# kth_largest: k_adj = (omq*(N-1))>>32 must equal 509 so second output is
# desc[510] (511th largest value).
_OMQ = 1069052418
KTH_Q = 1.0 - _OMQ / 4294967296.0
GELU_MODE = "act"  # "act" (HW table) or "manual" (exact jax tanh formula)

# const block columns
C_ID, C_ONES, C_IOTA, C_REP, C_SEL, C_IDB, C_END = 0, 128, 272, 400, 528, 536, 600
# packed input-vector block columns
V_WR, V_L1G, V_L1B, V_L2G, V_L2B, V_BPJ, V_B1, V_B2 = (
    0, 24, 30, 36, 42, 48, 54, 78)

PHASE_MARKS = []  # (phase_name, first_instruction_index) — analysis only


def mark(nc, name):
    import re
    nxt = nc.get_next_instruction_name()
    m = re.search(r"(\d+)$", nxt)
    PHASE_MARKS.append((name, int(m.group(1)) if m else 0))


def ts(i, n):
    return slice(i * n, (i + 1) * n)


def mmr(nc, out, lhsT, rhs, **kw):
    """fp32r matmul (full-rate for free dim >= 256); operands must be
    F32R-declared tiles."""
    nc.tensor.matmul(out, lhsT, rhs, **kw)


def trr(nc, out, in_, identr, **kw):
    """PE transpose in f32r mode (1.5 cyc/row vs 2.0 for fp32).
    in_ and identr must be F32R-declared tiles."""
    nc.tensor.transpose(out.bitcast(F32R), in_, identr, **kw)


def trb(nc, out, in_, identb, **kw):
    """PE transpose of bf16 data (1.0 cyc/row); out must be a BF16 PSUM
    tile, in_ and identb BF16-declared."""
    nc.tensor.transpose(out, in_, identb, **kw)


def emit(nc, tc, dr, ctx):
    x_d, out_d = dr["x_d"], dr["out_d"]
    pr_d, gat_d = dr["pr_d"], dr["gat_d"]

    cpool = ctx.enter_context(tc.tile_pool(name="consts", bufs=1))
    cblk = cpool.tile([128, C_END], F32, tag="cblk")
    nc.sync.dma_start(cblk[:], dr["cst_d"][:])
    ident = cblk[:, C_ID:C_ID + 128]
    ones_col = cblk[:, C_ONES:C_ONES + 1]
    ones_row = cblk[0:1, C_ONES:C_ONES + 144]
    iota_row = cblk[0:16, C_IOTA:C_IOTA + 128]
    rep16 = cblk[0:16, C_REP:C_REP + 128]
    sel2 = cblk[0:8, C_SEL:C_SEL + E]
    identr = cpool.tile([128, 128], F32R, tag="identr")
    nc.sync.dma_start(identr[:], dr["cst_d"][:, 0:128].bitcast(F32R))
    identb = cpool.tile([128, 128], BF16, tag="identb")
    nc.sync.dma_start(identb[:],
                      dr["cst_d"][:, C_IDB:C_IDB + 64].bitcast(BF16))
    onesr_r = cpool.tile([1, 140], F32R, tag="onesr_r")
    nc.sync.dma_start(onesr_r[:],
                      dr["cst_d"][0:1, C_ONES:C_ONES + 140].bitcast(F32R))

    vec = cpool.tile([128, 84], F32, tag="vec")
    nc.sync.dma_start(vec[:], dr["vec_d"][:])
    wr24 = vec[:, V_WR:V_WR + 24]
    ln1g_v = vec[:, V_L1G:V_L1G + KD]
    ln1b_v = vec[:, V_L1B:V_L1B + KD]
    ln2g = vec[:, V_L2G:V_L2G + KD]
    ln2b = vec[:, V_L2B:V_L2B + KD]
    bproj = vec[:, V_BPJ:V_BPJ + KD]
    b1sb = vec[:, V_B1:V_B1 + KH]
    b2sb = vec[:, V_B2:V_B2 + KD]

    xn_sb = cpool.tile([128, NT, D], BF16, tag="xn_sb")
    probsT = cpool.tile([E, N], F32, tag="probsT")

    # Weight pools live for the whole kernel so their SBUF is never reused
    # by other pools (prefetch DMAs have no deps and schedule early).
    wpool = ctx.enter_context(tc.tile_pool(name="w12", bufs=1))
    w1_sb = wpool.tile([128, KD, HID], FP8, tag="w1sb")
    w2_sb = wpool.tile([128, KH, D], FP8, tag="w2sb")
    wqkv = wpool.tile([128, KD, 3 * D], FP8, tag="wqkv")
    wproj = wpool.tile([128, KD, D], FP8, tag="wproj")
    nc.gpsimd.dma_start(
        wqkv[:],
        bass.AP(dr["wqkv_d"], 0,
                [[3 * D, 128], [128 * 3 * D, KD], [1, 3 * D]]))
    nc.gpsimd.dma_start(
        wproj[:],
        bass.AP(dr["wproj_d"], 0, [[D, 128], [128 * D, KD], [1, D]]))
    for _k in range(KD):
        nc.gpsimd.dma_start(w1_sb[:, _k, :], dr["w1_d"][ts(_k, 128), :])
    for _t4 in range(4):
        nc.gpsimd.dma_start(
            w2_sb[:, ts(_t4, 6), :],
            bass.AP(dr["w2_d"], _t4 * 6 * 128 * D,
                    [[D, 128], [128 * D, 6], [1, D]]))

    # ---------------- P1: stream x -> ln1/xn16, x^T, logits, softmax -------
    mark(nc, "P1_xT_router")

    with (
        tc.tile_pool(name="xt", bufs=2) as xt_pool,
        tc.tile_pool(name="pst", bufs=1, space="PSUM") as pst_pool,
        tc.tile_pool(name="psr", bufs=1, space="PSUM") as psr_pool,
    ):
        for g in range(4):
            x_g = xt_pool.tile([128, 4, D], F32, tag="x_g", bufs=3,
                               name="x_g")
            nc.sync.dma_start(
                x_g[:],
                bass.AP(x_d, g * 512 * D, [[D, 128], [128 * D, 4], [1, D]]))
            # --- LN1 stats + rescale -> bf16 staging ---
            rs_, nmr_ = [], []
            for t in range(4):
                st = xt_pool.tile([128, 12], F32, tag="ln_st", bufs=2,
                                  name="ln_st")
                nc.vector.bn_stats(st[:, 0:6], x_g[:, t, 0:384])
                nc.vector.bn_stats(st[:, 6:12], x_g[:, t, 384:768])
                ag = xt_pool.tile([128, 2], F32, tag="ln_ag", bufs=8,
                                  name="ln_ag")
                nc.vector.bn_aggr(ag[:], st[:])
                ve = xt_pool.tile([128, 2], F32, tag="ln_ve", bufs=8,
                                  name="ln_ve")
                nc.vector.tensor_scalar(ve[:, 0:1], ag[:, 1:2], EPS, None,
                                        op0=ALU.add)
                nc.vector.reciprocal(ve[:, 1:2], ve[:, 0:1])
                rs_.append(ve)
                nmr_.append(ag)
            r4 = xt_pool.tile([128, 8], F32, tag="ln_r4", bufs=2, name="ln_r4")
            for t in range(4):
                nc.scalar.activation(r4[:, 2 * t:2 * t + 1], rs_[t][:, 1:2],
                                     AF.Sqrt)
            for t in range(4):
                nc.vector.scalar_tensor_tensor(r4[:, 2 * t + 1:2 * t + 2],
                                               nmr_[t][:, 0:1], -1.0,
                                               r4[:, 2 * t:2 * t + 1],
                                               op0=ALU.mult, op1=ALU.mult)
            for t in range(4):
                nc.scalar.activation(xn_sb[:, g * 4 + t, :], x_g[:, t, :],
                                     AF.Identity,
                                     bias=r4[:, 2 * t + 1:2 * t + 2],
                                     scale=r4[:, 2 * t:2 * t + 1])
            # --- x^T via exact fp32 transposes (router is tie-sensitive) ---
            xTc = xt_pool.tile([128, KD, 512], F32, tag="xTc", bufs=2,
                               name="xTc")
            pss = [pst_pool.tile([128, 512], F32, tag=f"pst{k}",
                                 name=f"pst{k}") for k in range(KD)]
            for t in range(4):
                for k in range(KD):
                    nc.tensor.transpose(pss[k][:, ts(t, 128)],
                                        x_g[:, t, ts(k, 128)], ident)
            for k in range(KD):
                if k % 2 == 0:
                    nc.vector.tensor_copy(xTc[:, k, :], pss[k][:])
                else:
                    nc.scalar.activation(xTc[:, k, :], pss[k][:], AF.Identity)
            # --- logits token-major (moving operand = Wr, free dim E) ---
            ps_lg = psr_pool.tile([128, 4 * E], F32, tag="small1",
                                  name="ps_lg")
            for t in range(4):
                for k in range(KD):
                    nc.tensor.matmul(ps_lg[:, ts(t, E)], xTc[:, k, ts(t, 128)],
                                     wr24[:, ts(k, E)],
                                     start=(k == 0), stop=(k == KD - 1))
            sb_lg = xt_pool.tile([128, 4 * E], F32, tag="sb_lg", bufs=2,
                                 name="sb_lg")
            nc.vector.tensor_copy(sb_lg[:], ps_lg[:])
            lgT = psr_pool.tile([E, 512], F32, tag="small2", name="lgT")
            for t in range(4):
                nc.tensor.transpose(lgT[:, ts(t, 128)], sb_lg[:, ts(t, E)],
                                    ident)
            # --- softmax over experts for this group ---
            expg = xt_pool.tile([E, 512], F32, tag="expg", bufs=2,
                                name="expg")
            nc.scalar.activation(expg[:], lgT[:], AF.Exp)
            z = psr_pool.tile([1, 512], F32, tag="small1", name="z")
            nc.tensor.matmul(z[:], ones_col[0:E, :], expg[:],
                             start=True, stop=True)
            rzg = xt_pool.tile([1, 512], F32, tag="rzg", bufs=2, name="rzg")
            nc.vector.reciprocal(rzg[:], z[:])
            rzb = psr_pool.tile([E, 512], F32, tag="small2", name="rzb")
            nc.tensor.matmul(rzb[:], ones_row[0:1, 0:E], rzg[:],
                             start=True, stop=True)
            nc.vector.tensor_tensor(probsT[:, ts(g, 512)], expg[:],
                                    rzb[:], ALU.mult)
            nc.gpsimd.dma_start(pr_d[:, ts(g, 512)], probsT[:, ts(g, 512)])

    # ---------------- P2: thresholds + compaction --------------------------
    mark(nc, "P2_router_topk")

    idx_sb, gates_tm = [], []
    with (
        tc.tile_pool(name="r2", bufs=1) as r2,
        tc.tile_pool(name="psz", bufs=2, space="PSUM") as psz,
        tc.tile_pool(name="psb", bufs=2, space="PSUM") as psb,
    ):
        ptm = r2.tile([128, E, 16], F32, tag="ptm")
        nc.sync.dma_start(ptm[:],
                          bass.AP(pr_d, 0, [[16, 128], [N, E], [1, 16]]))
        kth = r2.tile([1, 2 * E], F32, tag="kth")
        for e in range(E):
            nc.gpsimd.kth_largest(kth[:, ts(e, 2)], ptm[:, e, :],
                                  n_per_lane=16, k=510, quantile=KTH_Q)
        # threshold column [E,1] = sel2.T @ (kth row transposed); no DRAM trip
        kthT = psz.tile([2 * E, 1], F32, tag="z1", name="kthT")
        nc.tensor.transpose(kthT[:], kth[:], ident[0:1, 0:1])
        kthT_s = r2.tile([2 * E, 1], F32, tag="kthT_s")
        nc.vector.tensor_copy(kthT_s[:], kthT[:])
        kv = psz.tile([E, 1], F32, tag="z1", name="kv")
        nc.tensor.matmul(kv[:], sel2, kthT_s[:], start=True, stop=True)
        pm = r2.tile([E, N], F32, tag="pm")
        nc.vector.scalar_tensor_tensor(pm[:], probsT[:], kv[:, 0:1], probsT[:],
                                       op0=ALU.is_lt, op1=ALU.mult)
        v2 = r2.tile([E, 1], F32, tag="v2")
        nc.vector.tensor_reduce(v2[:], pm[:], axis=AX.X, op=ALU.max)
        # broadcast v2 to [16, E] on-chip
        v2T = psz.tile([1, E], F32, tag="z1", name="v2T")
        nc.tensor.transpose(v2T[:], v2[:], ident[0:E, 0:E])
        v2r = r2.tile([1, E], F32, tag="v2r")
        nc.vector.tensor_copy(v2r[:], v2T[:])
        v2wp = psz.tile([16, E], F32, tag="z1", name="v2wp")
        nc.tensor.matmul(v2wp[:], ones_row[0:1, 0:16], v2r[:],
                         start=True, stop=True)
        v2w = r2.tile([16, E], F32, tag="v2w")
        nc.vector.tensor_copy(v2w[:], v2wp[:])

        for e in range(E):
            pw = r2.tile([16, 128], F32, tag="pw", bufs=2, name="pw")
            nc.sync.dma_start(pw[:],
                              bass.AP(pr_d, e * N, [[128, 16], [1, 128]]))
            mw = r2.tile([16, 128], F32, tag="mw", bufs=2, name="mw")
            nc.vector.tensor_scalar(mw[:], pw[:], v2w[:, e:e + 1], None,
                                    op0=ALU.is_ge)
            tidx = r2.tile([16, 128], F32, tag="tidx", bufs=2)
            nc.vector.tensor_tensor(tidx[:], mw[:], iota_row, ALU.mult)
            nc.vector.tensor_scalar(tidx[:], tidx[:], 1.0, None,
                                    op0=ALU.subtract)
            idx_c = r2.tile([16, 32], F32, tag="idx_c", bufs=2)
            nf = r2.tile([1, 1], U32, tag="nf", bufs=2)
            nc.gpsimd.sparse_gather(idx_c[:], tidx[:], num_found=nf[:])
            # replicate [16,32] -> [128,32] via PE (idx values exact in fp32)
            repp = psb.tile([128, 32], F32, tag="rep", name="repp")
            nc.tensor.matmul(repp[:], rep16, idx_c[:], start=True, stop=True)
            isb = cpool.tile([128, 32], I16, tag=f"idx_sb{e}",
                             name=f"idx_sb{e}")
            nc.vector.tensor_copy(isb[:], repp[:])
            idx_sb.append(isb)
            gw = r2.tile([16, 128], F32, tag="gw", bufs=2)
            nc.vector.tensor_tensor(gw[:], mw[:], pw[:], ALU.mult)
            nc.vector.scalar_tensor_tensor(gw[:], mw[:], 1.0, gw[:],
                                           op0=ALU.subtract, op1=ALU.add)
            gat_c = r2.tile([16, 32], F32, tag="gat_c", bufs=2)
            nf2 = r2.tile([1, 1], U32, tag="nf2", bufs=2)
            nc.gpsimd.sparse_gather(gat_c[:], gw[:], num_found=nf2[:])
            nc.sync.dma_start(gat_d[e:e + 1, :], gat_c[:])
            gtm = cpool.tile([128, 4], F32, tag=f"gates{e}", name=f"gates{e}")
            nc.sync.dma_start(
                gtm[:], bass.AP(gat_d, e * 512, [[1, 8], [32, 16], [8, 4]]))
            gates_tm.append(gtm)
        for g in range(4):
            nc.gpsimd.dma_start(out_d[ts(g, 512), :], x_d[ts(g, 512), :])

    # ---------------- shared helpers ----------------
    def ln_tiles(xg, g_sb, b_sb, dstT, e, xp, psp, pstag):
        """LayerNorm over gathered fp32 token rows -> feature-major dstT
        (bf16), gamma/beta folded into the post-transpose activation."""
        kde = KDE[e]
        rs_, nmr_ = [], []
        for t in range(4):
            st = xp.tile([128, 12], F32, tag="ln_st", name="ln_st")
            nc.vector.bn_stats(st[:, 0:6], xg[:, t, 0:384])
            nc.vector.bn_stats(st[:, 6:12], xg[:, t, 384:768])
            ag = xp.tile([128, 2], F32, tag="ln_ag", bufs=4, name="ln_ag")
            nc.vector.bn_aggr(ag[:], st[:])
            ve = xp.tile([128, 2], F32, tag="ln_ve", bufs=4, name="ln_ve")
            nc.vector.tensor_scalar(ve[:, 0:1], ag[:, 1:2], EPS, None,
                                    op0=ALU.add)
            nc.vector.reciprocal(ve[:, 1:2], ve[:, 0:1])
            rs_.append(ve)
            nmr_.append(ag)
        r4 = xp.tile([128, 8], F32, tag="ln_r4", name="ln_r4")
        for t in range(4):
            nc.scalar.activation(r4[:, 2 * t:2 * t + 1], rs_[t][:, 1:2],
                                 AF.Sqrt)
        for t in range(4):
            nc.vector.scalar_tensor_tensor(r4[:, 2 * t + 1:2 * t + 2],
                                           nmr_[t][:, 0:1], -1.0,
                                           r4[:, 2 * t:2 * t + 1],
                                           op0=ALU.mult, op1=ALU.mult)
        xr = xp.tile([128, 4, D], BF16, tag="ln_xr", bufs=2, name="ln_xr")
        for t in range(4):
            if t % 2 == 0:
                nc.scalar.activation(xr[:, t, :], xg[:, t, :], AF.Identity,
                                     bias=r4[:, 2 * t + 1:2 * t + 2],
                                     scale=r4[:, 2 * t:2 * t + 1])
            else:
                nc.vector.tensor_scalar(xr[:, t, :], xg[:, t, :],
                                        r4[:, 2 * t:2 * t + 1],
                                        r4[:, 2 * t + 1:2 * t + 2],
                                        op0=ALU.mult, op1=ALU.add)
        for k in range(kde):
            kp = min(128, DE[e] - k * 128)
            ps = psp.tile([128, 512], BF16, tag=pstag, name="ps_ln")
            for t in range(4):
                trb(nc, ps[0:kp, ts(t, 128)],
                    xr[:, t, k * 128:k * 128 + kp], identb[0:128, 0:128])
            nc.vector.tensor_scalar(dstT[0:kp, k, :], ps[0:kp, :],
                                    g_sb[0:kp, k:k + 1], b_sb[0:kp, k:k + 1],
                                    op0=ALU.mult, op1=ALU.add)

    def out_transpose_scatter(yT, e, xp, psp, pstag, ytag, ybufs):
        kde = KDE[e]
        dpad = DPAD[e]
        ytok = xp.tile([128, 4, dpad], F32, tag=ytag, bufs=ybufs, name="ytok")
        if dpad > DE[e]:
            nc.vector.memset(ytok[:, :, DE[e]:dpad], 0.0)
        for k in range(kde):
            kp = min(128, DE[e] - k * 128)
            ps = psp.tile([128, 512], BF16, tag=pstag, name="ps_ot")
            for t in range(4):
                trb(nc, ps[:, t * 128:t * 128 + kp],
                    yT[0:kp, k, ts(t, 128)], identb[0:kp, 0:kp])
            for t in range(4):
                nc.vector.tensor_scalar(ytok[:, t, k * 128:k * 128 + kp],
                                        ps[:, t * 128:t * 128 + kp],
                                        gates_tm[e][:, t:t + 1], None,
                                        op0=ALU.mult)
        nc.gpsimd.dma_scatter_add(out_d[:, 0:dpad], ytok[:], idx_sb[e][:],
                                  CAP, CAP, dpad, elem_step=D)

    # ---------------- P3: attention branch ----------------
    mark(nc, "P3_attn")

    with (
        tc.tile_pool(name="ax", bufs=1) as ax_pool,
        tc.tile_pool(name="aw", bufs=2) as aw_pool,
        tc.tile_pool(name="psA", bufs=2, space="PSUM") as psA,
        tc.tile_pool(name="psS", bufs=4, space="PSUM") as psS,
        tc.tile_pool(name="psV", bufs=2, space="PSUM") as psV,
    ):
        for e in range(E):
            kde, de = KDE[e], DE[e]
            xeT = ax_pool.tile([128, KD, 512], BF16, tag="xeT", bufs=2,
                               name="xeT")
            nc.gpsimd.dma_gather(xeT[:, 0:(DPAD[e] // 128), :],
                                 xn_sb[:], idx_sb[e][:],
                                 CAP, CAP, DPAD[e], transpose=True,
                                 sbuf_tokens_per_rank=128,
                                 sbuf_free_dim_per_rank=2 * D)
            xe8 = ax_pool.tile([128, KD, 512], FP8, tag="xe8", bufs=2,
                               name="xe8")
            for k in range(kde):
                kp = min(128, de - k * 128)
                nc.vector.tensor_scalar(xe8[0:kp, k, :], xeT[0:kp, k, :],
                                        ln1g_v[0:kp, k:k + 1],
                                        ln1b_v[0:kp, k:k + 1],
                                        op0=ALU.mult, op1=ALU.add)
            if e == 2:
                nc.vector.memset(xe8[64:128, 1, :], 0.0)

            qT = ax_pool.tile([128, KD, 512], BF16, tag="qT", bufs=2,
                              name="qT")
            kT = ax_pool.tile([128, KD, 512], BF16, tag="kT", bufs=2,
                              name="kT")
            v_sb = ax_pool.tile([128, 4, 12 * 65], BF16, tag="v_sb",
                                name="v_sb")
            for h, dh in HEADS_E[e]:
                nc.vector.memset(v_sb[:, :, h * 65 + dh:(h + 1) * 65], 1.0)
            for mk in range(kde):
                mw_ = min(128, de - mk * 128)
                for dst, coff in ((qT, 0), (kT, D)):
                    ps = psA.tile([128, 512], F32, tag="a", name="ps_qk")
                    pairs = kde // 2
                    for p in range(pairs):
                        nc.tensor.matmul(
                            ps[0:mw_, :],
                            wqkv[:, 2 * p:2 * p + 2,
                                 coff + mk * 128:coff + mk * 128 + mw_],
                            xe8[:, 2 * p:2 * p + 2, :], start=(p == 0),
                            stop=(p == pairs - 1 and kde % 2 == 0),
                            perf_mode=DR)
                    if kde % 2:
                        kp = de - (kde - 1) * 128
                        nc.tensor.matmul(
                            ps[0:mw_, :],
                            wqkv[0:kp, kde - 1,
                                 coff + mk * 128:coff + mk * 128 + mw_],
                            xe8[0:kp, kde - 1, :], start=(kde == 1),
                            stop=True)
                    nc.scalar.activation(dst[0:mw_, mk, :], ps[0:mw_, :],
                                         AF.Identity, scale=1.0 / W8SCALE)
            for t in range(4):
                for nsp in range((de + 511) // 512):
                    nw = min(512, de - nsp * 512)
                    ps = psV.tile([128, 512], F32, tag="v", name="ps_v")
                    pairs = kde // 2
                    for p in range(pairs):
                        nc.tensor.matmul(
                            ps[:, 0:nw], xe8[:, 2 * p:2 * p + 2, ts(t, 128)],
                            wqkv[:, 2 * p:2 * p + 2,
                                 2 * D + nsp * 512:2 * D + nsp * 512 + nw],
                            start=(p == 0),
                            stop=(p == pairs - 1 and kde % 2 == 0),
                            perf_mode=DR)
                    if kde % 2:
                        kp = de - (kde - 1) * 128
                        nc.tensor.matmul(
                            ps[:, 0:nw], xe8[0:kp, kde - 1, ts(t, 128)],
                            wqkv[0:kp, kde - 1,
                                 2 * D + nsp * 512:2 * D + nsp * 512 + nw],
                            start=(kde == 1), stop=True)
                    if nw % DH == 0:
                        h0 = (nsp * 512) // DH
                        nh = nw // DH
                        vdst = v_sb[:, t, h0 * 65:(h0 + nh) * 65].rearrange(
                            "p (h c) -> p h c", c=65)[:, :, 0:DH]
                        vsrc = ps[:, 0:nw].rearrange("p (h c) -> p h c", c=DH)
                        nc.vector.tensor_scalar(vdst, vsrc,
                                                1.0 / W8SCALE, None,
                                                op0=ALU.mult)
                    else:
                        for h, dh in HEADS_E[e]:
                            lo = h * DH
                            if lo >= nsp * 512 + nw or lo + dh <= nsp * 512:
                                continue
                            nc.vector.tensor_scalar(
                                v_sb[:, t, h * 65:h * 65 + dh],
                                ps[:, lo - nsp * 512:lo - nsp * 512 + dh],
                                1.0 / W8SCALE, None, op0=ALU.mult)

            o_sb = ax_pool.tile([128, KD, 512], FP8, tag="o_sb", name="o_sb")
            if e == 2:
                nc.vector.memset(o_sb[64:128, 1, :], 0.0)
            e_sb = ax_pool.tile([128, 4, 512], BF16, tag="e_sb",
                                bufs=3, name="e_sb")
            for h, dh in HEADS_E[e]:
                mk, off = (h * DH) // 128, (h * DH) % 128
                for kc in range(4):
                    sps = psS.tile([128, 512], F32, tag="s", name="ps_s")
                    nc.tensor.matmul(sps[:],
                                     kT[off:off + dh, mk, ts(kc, 128)],
                                     qT[off:off + dh, mk, :],
                                     start=True, stop=True)
                    nc.scalar.activation(e_sb[:, kc, :], sps[:], AF.Exp,
                                         scale=float(DH ** -0.5))
                oa = psV.tile([128, 512], F32, tag="v", name="ps_oa")
                for kc in range(4):
                    nc.tensor.matmul(oa[0:dh + 1, :],
                                     v_sb[:, kc, h * 65:h * 65 + dh + 1],
                                     e_sb[:, kc, :], start=(kc == 0),
                                     stop=(kc == 3))
                rs = aw_pool.tile([1, 512], BF16, tag="rs", bufs=2,
                                  name="rs")
                nc.vector.reciprocal(rs[:], oa[dh:dh + 1, :])
                rb_sb = aw_pool.tile([64, 512], BF16, tag="rb_sb", bufs=2,
                                     name="rb_sb")
                nc.gpsimd.partition_broadcast(rb_sb[0:dh, :], rs[:])
                if off == 0:
                    nc.vector.tensor_tensor(o_sb[0:dh, mk, :], oa[0:dh, :],
                                            rb_sb[0:dh, :], ALU.mult)
                else:
                    on = aw_pool.tile([64, 512], FP8, tag="on", name="on")
                    nc.vector.tensor_tensor(on[0:dh, :], oa[0:dh, :],
                                            rb_sb[0:dh, :], ALU.mult)
                    nc.sync.dma_start(o_sb[off:off + dh, mk, :], on[0:dh, :])
            yeT = ax_pool.tile([128, KD, 512], BF16, tag="yeT", bufs=1,
                               name="yeT")
            for mk in range(kde):
                mw_ = min(128, de - mk * 128)
                ps = psA.tile([128, 512], F32, tag="a", name="ps_pr")
                pairs = kde // 2
                for p in range(pairs):
                    nc.tensor.matmul(ps[0:mw_, :],
                                     wproj[:, 2 * p:2 * p + 2,
                                           mk * 128:mk * 128 + mw_],
                                     o_sb[:, 2 * p:2 * p + 2, :],
                                     start=(p == 0),
                                     stop=(p == pairs - 1 and kde % 2 == 0),
                                     perf_mode=DR)
                if kde % 2:
                    kp = de - (kde - 1) * 128
                    nc.tensor.matmul(ps[0:mw_, :],
                                     wproj[0:kp, kde - 1,
                                           mk * 128:mk * 128 + mw_],
                                     o_sb[0:kp, kde - 1, :],
                                     start=(kde == 1), stop=True)
                nc.vector.tensor_scalar(yeT[0:mw_, mk, :], ps[0:mw_, :],
                                        1.0 / W8SCALE,
                                        bproj[0:mw_, mk:mk + 1],
                                        op0=ALU.mult, op1=ALU.add)
            out_transpose_scatter(yeT, e, ax_pool, psS, "s", "ytok", 1)

    # ---------------- P4: MLP branch ----------------
    mark(nc, "P4_mlp")

    with (
        tc.tile_pool(name="mx", bufs=1) as mx_pool,
        tc.tile_pool(name="mw", bufs=2) as mw_pool,
        tc.tile_pool(name="psM", bufs=2, space="PSUM") as psM,
        tc.tile_pool(name="psY", bufs=1, space="PSUM") as psY,
    ):
        xeTs = [None] * E
        for e in (0, 1, 2, 3):
            xg = mx_pool.tile([128, 4, D], F32, tag="xg2", bufs=2, name="xg2")
            nc.gpsimd.dma_gather(xg[:], out_d[:], idx_sb[e][:], CAP, CAP, D)
            xeT = mx_pool.tile([128, KDE[e], 512], FP8, tag=f"xe2T{e}",
                               name=f"xe2T{e}")
            ln_tiles(xg, ln2g, ln2b, xeT, e, mw_pool, psM, "m")
            if e == 2:
                # zero junk rows of the partial last k-tile so DoubleRow
                # pairs contract over clean zeros
                nc.vector.memset(xeT[64:128, KDE[e] - 1, :], 0.0)
            xeTs[e] = xeT
        for e in (0, 1, 2, 3):
            kde, de, khe = KDE[e], DE[e], KHE[e]
            xeT = xeTs[e]
            h_all = mx_pool.tile([128, KH, 512], FP8, tag="h_all",
                                 bufs=2, name="h_all")
            for th in range(khe):
                hps = psM.tile([128, 512], F32, tag="m", name="ps_h")
                pairs = kde // 2
                for p in range(pairs):
                    nc.tensor.matmul(hps[:],
                                     w1_sb[:, 2 * p:2 * p + 2, ts(th, 128)],
                                     xeT[:, 2 * p:2 * p + 2, :],
                                     start=(p == 0),
                                     stop=(p == pairs - 1 and kde % 2 == 0),
                                     perf_mode=DR)
                if kde % 2:
                    kp = de - (kde - 1) * 128
                    nc.tensor.matmul(hps[:], w1_sb[0:kp, kde - 1, ts(th, 128)],
                                     xeT[0:kp, kde - 1, :],
                                     start=(kde == 1), stop=True)
                if GELU_MODE == "act":
                    nc.scalar.activation(h_all[:, th, :], hps[:],
                                         AF.Gelu_apprx_tanh,
                                         bias=b1sb[:, th:th + 1],
                                         scale=1.0 / W8SCALE)
                else:
                    u = mw_pool.tile([128, 512], F32, tag="g_u", name="g_u")
                    nc.scalar.activation(u[:], hps[:], AF.Identity,
                                         bias=b1sb[:, th:th + 1],
                                         scale=1.0 / W8SCALE)
                    t1 = mw_pool.tile([128, 512], F32, tag="g_t1", name="g_t1")
                    nc.vector.tensor_tensor(t1[:], u[:], u[:], ALU.mult)
                    nc.vector.tensor_tensor(t1[:], t1[:], u[:], ALU.mult)
                    nc.vector.scalar_tensor_tensor(t1[:], t1[:], 0.044715,
                                                   u[:], op0=ALU.mult,
                                                   op1=ALU.add)
                    nc.scalar.activation(t1[:], t1[:], AF.Tanh,
                                         scale=0.7978845608028654)
                    nc.vector.scalar_tensor_tensor(t1[:], t1[:], 1.0, u[:],
                                                   op0=ALU.add, op1=ALU.mult)
                    nc.vector.tensor_scalar(h_all[:, th, :], t1[:], 0.5,
                                            None, op0=ALU.mult)
            y2T = mx_pool.tile([128, KD, 512], BF16, tag="y2T", name="y2T")
            for mk in range(kde):
                mw_ = min(128, de - mk * 128)
                yp = psY.tile([128, 512], F32, tag="yy", bufs=2, name="ps_y")
                hpairs = khe // 2
                for q in range(hpairs):
                    nc.tensor.matmul(yp[0:mw_, :],
                                     w2_sb[:, 2 * q:2 * q + 2,
                                           mk * 128:mk * 128 + mw_],
                                     h_all[:, 2 * q:2 * q + 2, :],
                                     start=(q == 0),
                                     stop=(q == hpairs - 1 and khe % 2 == 0),
                                     perf_mode=DR)
                if khe % 2:
                    nc.tensor.matmul(yp[0:mw_, :],
                                     w2_sb[:, khe - 1,
                                           mk * 128:mk * 128 + mw_],
                                     h_all[:, khe - 1, :],
                                     start=(khe == 1), stop=True)
                nc.vector.tensor_scalar(y2T[0:mw_, mk, :], yp[0:mw_, :],
                                        1.0 / W8SCALE,
                                        b2sb[0:mw_, mk:mk + 1],
                                        op0=ALU.mult, op1=ALU.add)
            out_transpose_scatter(y2T, e, mx_pool, psM, "m", "xg2", 2)


def build_nc():
    nc = bacc.Bacc("TRN2", target_bir_lowering=False, debug=False)
    dr = {}
    dr["x_d"] = nc.dram_tensor("x", [N, D], F32, kind="ExternalInput")
    dr["wqkv_d"] = nc.dram_tensor("Wqkv", [D, 3 * D], FP8,
                                  kind="ExternalInput")
    dr["wproj_d"] = nc.dram_tensor("Wproj", [D, D], FP8,
                                   kind="ExternalInput")
    dr["w1_d"] = nc.dram_tensor("W1", [D, HID], FP8, kind="ExternalInput")
    dr["w2_d"] = nc.dram_tensor("W2", [HID, D], FP8, kind="ExternalInput")
    dr["cst_d"] = nc.dram_tensor("c_blk", [128, C_END], F32,
                                 kind="ExternalInput")
    dr["vec_d"] = nc.dram_tensor("c_vec", [128, 84], F32,
                                 kind="ExternalInput")
    dr["out_d"] = nc.dram_tensor("out", [N, D], F32, kind="ExternalOutput")
    dr["pr_d"] = nc.dram_tensor("pr_stage", [E, N], F32)
    dr["gat_d"] = nc.dram_tensor("gat_stage", [E, 512], F32)

    from contextlib import ExitStack
    with tile.TileContext(nc) as tc, ExitStack() as ctx, \
            nc.allow_low_precision(reason="bf16/fp32r rounding is intentional"):
        emit(nc, tc, dr, ctx)
    nc.compile()
    return nc


def make_consts():
    c = np.zeros((128, C_END), np.float32)
    c[:, C_ID:C_ID + 128] = np.eye(128, dtype=np.float32)
    c[:, C_ONES:C_ONES + 144] = 1.0
    c[0:16, C_IOTA:C_IOTA + 128] = (
        np.arange(16)[:, None] * 128 + np.arange(128)[None, :] + 1)
    c[0:16, C_REP:C_REP + 128] = (
        np.arange(128)[None, :] % 16 == np.arange(16)[:, None])
    for e in range(E):
        c[2 * e + 1, C_SEL + e] = 1.0
    import ml_dtypes
    idb = np.eye(128, dtype=ml_dtypes.bfloat16)
    c[:, C_IDB:C_IDB + 64] = idb.view(np.uint16).view(np.float32)
    return {"c_blk": c}


def make_vecs(inputs):
    v = np.zeros((128, 84), np.float32)
    wr = np.asarray(inputs["Wr"], np.float32)
    v[:, V_WR:V_WR + 24] = (
        wr.reshape(KD, 128, E).transpose(1, 0, 2).reshape(128, 24))
    for off, nm in ((V_L1G, "ln1_g"), (V_L1B, "ln1_b"), (V_L2G, "ln2_g"),
                    (V_L2B, "ln2_b"), (V_BPJ, "bproj")):
        v[:, off:off + KD] = (
            np.asarray(inputs[nm], np.float32).reshape(KD, 128).T)
    v[:, V_B1:V_B1 + KH] = (
        np.asarray(inputs["b1"], np.float32).reshape(KH, 128).T)
    v[:, V_B2:V_B2 + KD] = (
        np.asarray(inputs["b2"], np.float32).reshape(KD, 128).T)
    return v


def make_in_maps(inputs):
    import ml_dtypes
    shared = {}
    for k in ["Wqkv", "Wproj", "W1", "W2"]:
        a = (np.asarray(inputs[k], np.float32) * W8SCALE).astype(
            ml_dtypes.float8_e4m3)
        shared[k] = np.ascontiguousarray(a)
    shared["c_vec"] = make_vecs(inputs)
    shared.update(make_consts())
    x = np.asarray(inputs["x"], np.float32)
    in_maps = []
    for b in range(B):
        m = {"x": np.ascontiguousarray(x[b])}
        m.update(shared)
        in_maps.append(m)
    return in_maps


_NC_CACHE = None


def kernel(**inputs):
    global _NC_CACHE
    if _NC_CACHE is None:
        _NC_CACHE = build_nc()
    nc = _NC_CACHE
    in_maps = make_in_maps(inputs)
    res = run_bass_kernel_spmd(nc, in_maps, core_ids=list(range(B)))
    return np.stack([r["out"] for r in res.results], axis=0)
